# revision 1
# baseline (speedup 1.0000x reference)
"""GCMC GraphConv kernel for 8 Trainium2 NeuronCores.

Computation:  out = ci * segment_sum((input_feat @ weight * cj)[src], dst)

Strategy v2 (aggregate-then-transform, dst-sharded, no collectives):
  - Algebra: out = ci * (A^T (cj*X)) W  — the dense transform W commutes with
    the segment-sum, so we aggregate raw (cj-scaled) X rows per destination
    and multiply by W once per dst block at the end.  This removes the
    X@W pre-pass and its HBM h round-trip entirely; the per-edge gather
    reads bf16 X' rows (256B, all 128 input features) straight from HBM.
  - Nodes are 1D-partitioned by destination: core c owns 98 dst blocks of
    128 rows, assigned by sorted edge count for balance.  Each edge is
    routed (on host) to the core owning its destination.
  - The per-edge gather uses SWDGE dma_gather with queue_num cycling over
    4 queues: calls on different queues run concurrently on different
    GPSIMD Q7 core pairs (~2.2ns/desc aggregate vs 6.35ns serial).
  - Scatter is a one-hot matmul accumulated TRANSPOSED:
        psT[feat, dst] += msg[e, feat]^T-contract oh[e, dst]
    (lhsT=msg, rhs=onehot), flushed into accT [128 feat, 98*128 dst] f32.
    One-hots are built in batches (one DVE is_equal per gather chunk).
  - Epilogue per dst block: matmul(lhsT=accT block, rhs=W) -> [128 dst, 64],
    scale by ci, store.
  - Edges are host-sorted by (src window of 25000, dst block); per-(w,slot)
    groups are padded to the max over cores (shared SPMD program); pad
    edges gather row 0 and carry dstb = -1 (no one-hot match -> zero).
"""

import dataclasses
import math

import numpy as np
import ml_dtypes

import concourse.bacc as bacc
import concourse.mybir as mybir
import concourse.tile as tile
from concourse.bass_utils import run_bass_kernel_spmd

BF16 = ml_dtypes.bfloat16
P = 128
NCORES = 8
D_IN = 128


@dataclasses.dataclass(frozen=True)
class Cfg:
    N: int = 100000
    D_OUT: int = 64
    NWIN: int = 4            # src windows; N/NWIN must be < 32768 (int16 idx)
    MAX_CHUNK_TILES: int = 8   # gather chunk (1024 descs = SWDGE ring capacity,
    #   a hard ucode limit: 12- and 16-tile chunks wedge the device even with
    #   a larger dynamic_dma_scratch_size)
    NQUEUES: int = 4         # SWDGE queues; round-robin -> concurrent desc-gen
    SCRATCH: int = 32768     # dynamic DMA descriptor carveout (bytes/partition)

    @property
    def n_loc(self):
        return self.N // NCORES

    @property
    def nblk(self):
        return math.ceil(self.n_loc / P)

    @property
    def win(self):
        return self.N // self.NWIN


CFG = Cfg()


# ---------------------------------------------------------------- host prep

def shard_edges(cfg: Cfg, src, dst):
    """Route and sort edges; build per-core padded index/dst arrays.

    Destination blocks are assigned to (core, slot) pairs by sorted edge
    count so each slot's 8 blocks have similar counts (the SPMD program
    pads every (w, slot) group to the max over its 8 cores).

    Returns (G, per_core, block_of):
      G[w][s]       tiles of (window w, slot s) — identical across cores
      per_core[c]   dict with idx{w} / dstb{w} device arrays
      block_of[c,s] global dst block handled by core c, slot s
    """
    nblk, win, nw_ = cfg.nblk, cfg.win, cfg.NWIN
    src = np.asarray(src, dtype=np.int64)
    dst = np.asarray(dst, dtype=np.int64)
    gb = dst >> 7                            # global dst block
    dstb = (dst & 127).astype(np.float32)    # dst within block
    wine = src // win
    src_loc = (src - wine * win).astype(np.int16)

    nblk_g = NCORES * nblk                   # padded global block count
    bc = np.bincount(gb, minlength=nblk_g)   # edges per global block
    order = np.argsort(-bc, kind="stable")   # blocks by descending count
    block_of = np.empty((NCORES, nblk), dtype=np.int64)
    block_core = np.empty(nblk_g, dtype=np.int64)
    block_slot = np.empty(nblk_g, dtype=np.int64)
    for s in range(nblk):
        grp = order[s * NCORES:(s + 1) * NCORES]
        block_of[:, s] = grp
        block_core[grp] = np.arange(NCORES)
        block_slot[grp] = s

    core = block_core[gb]
    slot = block_slot[gb]

    gid = (core * nw_ + wine) * nblk + slot
    counts = np.bincount(gid, minlength=NCORES * nw_ * nblk)
    counts = counts.reshape(NCORES, nw_, nblk)
    G = -(-counts.max(axis=0) // P)          # ceil tiles per (w, slot)
    tiles_w = G.sum(axis=1)                  # [NWIN]

    off_ws = np.zeros((nw_, nblk), dtype=np.int64)
    off_ws[:, 1:] = np.cumsum(G[:, :-1], axis=1) * P

    per_core = []
    for c in range(NCORES):
        m = core == c
        sl, db, we, bl = src_loc[m], dstb[m], wine[m], slot[m]
        # sort by (window, slot, src) — src-ordered within group for DRAM
        # row locality in the gather
        key = (we * nblk + bl) * (win + 1) + sl
        o = np.argsort(key, kind="stable")
        ks = (we[o] * nblk + bl[o])
        gcnt = np.bincount(ks, minlength=nw_ * nblk)
        gstart = np.concatenate([[0], np.cumsum(gcnt)[:-1]])
        within = np.arange(ks.size) - gstart[ks]
        wsel, ssel = ks // nblk, ks % nblk
        pos = off_ws[wsel, ssel] + within
        maps = {}
        for w in range(nw_):
            nw_edges = int(tiles_w[w]) * P
            ia = np.zeros(nw_edges, dtype=np.int16)         # pad -> row 0
            da = np.full(nw_edges, -1.0, dtype=np.float32)  # pad -> no match
            sel = wsel == w
            ia[pos[sel]] = sl[o][sel]
            da[pos[sel]] = db[o][sel]
            maps[f"idx{w}"] = np.ascontiguousarray(
                np.tile(ia.reshape(-1, 16).T, (8, 1)))
            maps[f"dstb{w}"] = np.ascontiguousarray(da.reshape(-1, P).T)
        per_core.append(maps)
    return G, per_core, block_of


def host_inputs(cfg: Cfg, input_feat, weight, cj, ci, block_of):
    """Shared (replicated) device inputs + per-core civ (slot layout)."""
    N, nblk = cfg.N, cfg.nblk
    # X' = cj * X, node-major bf16 (256B rows = dma_gather granularity)
    xs = (np.asarray(input_feat, dtype=np.float32)
          * np.asarray(cj, dtype=np.float32)).astype(BF16)
    xw = np.ascontiguousarray(xs)
    wgt = np.ascontiguousarray(np.asarray(weight, dtype=np.float32))
    iot = np.ascontiguousarray(
        np.broadcast_to(np.arange(P, dtype=np.float32), (P, P)).astype(BF16))
    # ci in (core, slot) layout: civ[c][p, s] = ci[block_of[c,s]*128 + p]
    cip = np.zeros(NCORES * nblk * P, dtype=np.float32)
    cif = np.asarray(ci, dtype=np.float32).reshape(-1)
    cip[:N] = cif
    cip = cip.reshape(NCORES * nblk, P)
    civs = [np.ascontiguousarray(cip[block_of[c]].T) for c in range(NCORES)]
    return {"xw": xw, "wgt": wgt, "iot": iot}, civs


# ---------------------------------------------------------------- device IR

def tile_blocks(cfg: Cfg, G, w):
    """Per-tile (block, k, g) for window w, in edge order."""
    out = []
    for b in range(cfg.nblk):
        g = int(G[w][b])
        for k in range(g):
            out.append((b, k, g))
    return out


def build_nc(cfg: Cfg, G):
    f32, bf16, i16 = mybir.dt.float32, mybir.dt.bfloat16, mybir.dt.int16
    dout, nblk, win = cfg.D_OUT, cfg.nblk, cfg.win
    tiles_w = [int(sum(G[w])) for w in range(cfg.NWIN)]

    nc = bacc.Bacc("TRN2", target_bir_lowering=False, debug=False,
                   num_swdge_queues=cfg.NQUEUES,
                   dynamic_dma_scratch_size=cfg.SCRATCH)
    xw = nc.dram_tensor("xw", [cfg.N, D_IN], bf16, kind="ExternalInput")
    wgt = nc.dram_tensor("wgt", [D_IN, dout], f32, kind="ExternalInput")
    civ = nc.dram_tensor("civ", [P, nblk], f32, kind="ExternalInput")
    iot = nc.dram_tensor("iot", [P, P], bf16, kind="ExternalInput")
    idx_t = [nc.dram_tensor(f"idx{w}", [P, tiles_w[w] * 8], i16,
                            kind="ExternalInput") for w in range(cfg.NWIN)]
    dstb_t = [nc.dram_tensor(f"dstb{w}", [P, tiles_w[w]], f32,
                             kind="ExternalInput") for w in range(cfg.NWIN)]
    out_t = nc.dram_tensor("out", [nblk * P, dout], f32, kind="ExternalOutput")

    with tile.TileContext(nc) as tc:
        with (
            tc.tile_pool(name="const", bufs=1) as cpool,
            tc.tile_pool(name="idx", bufs=2) as ipool,
            tc.tile_pool(name="msg", bufs=8) as mpool,
            tc.tile_pool(name="oh", bufs=8) as opool,
            tc.tile_pool(name="ps", bufs=4, space="PSUM") as pspool,
            tc.tile_pool(name="pso", bufs=4, space="PSUM") as psopool,
            tc.tile_pool(name="acc", bufs=1) as apool,
        ):
            wgt_sb = cpool.tile([P, dout], f32, tag="wgt")
            nc.sync.dma_start(out=wgt_sb[:], in_=wgt[:])
            iota_sb = cpool.tile([P, P], bf16, tag="iot")
            nc.sync.dma_start(out=iota_sb[:], in_=iot[:])
            ci_sb = cpool.tile([P, nblk], f32, tag="ci")
            nc.sync.dma_start(out=ci_sb[:], in_=civ[:])
            accT = apool.tile([P, nblk * P], f32, tag="accT")
            nc.vector.memset(accT[:], 0.0)
            out_sb = apool.tile([P, nblk * dout], f32, tag="out")

            st = {"ps": None}
            done_blocks = set()

            def emit_epilogue(b):
                """out_b = (accT_b)^T @ W, scaled by ci."""
                done_blocks.add(b)
                pso = psopool.tile([P, dout], f32, tag="pso")
                nc.tensor.matmul(
                    out=pso[:],
                    lhsT=accT[:, b * P:(b + 1) * P],
                    rhs=wgt_sb[:],
                    start=True, stop=True)
                nc.scalar.mul(
                    out_sb[:, b * dout:(b + 1) * dout],
                    pso[:],
                    ci_sb[:, b:b + 1])

            def emit_chunk(w, t0, tb, idx_sb, dst_sb, qn):
                """Gather one chunk of edges and matmul-scatter it."""
                t1 = min(t0 + cfg.MAX_CHUNK_TILES, len(tb))
                nt = t1 - t0
                ne = nt * P
                msg = mpool.tile([P, nt * D_IN], bf16, tag="msg")
                nc.gpsimd.dma_gather(
                    msg[:].rearrange("p (t f) -> p t f", f=D_IN),
                    xw[w * win:(w + 1) * win, :],
                    idx_sb[:, t0 * 8:t1 * 8],
                    ne, ne, D_IN,
                    queue_num=qn)
                oh = opool.tile([P, nt * P], bf16, tag="oh")
                nc.vector.tensor_tensor(
                    out=oh[:].rearrange("p (t n) -> p t n", n=P),
                    in0=dst_sb[:, t0:t1].rearrange("p (t o) -> p t o", o=1)
                        .to_broadcast([P, nt, P]),
                    in1=iota_sb[:].rearrange("p (o n) -> p o n", o=1)
                        .to_broadcast([P, nt, P]),
                    op=mybir.AluOpType.is_equal)
                for t in range(t0, t1):
                    b, k, g = tb[t]
                    if k == 0:
                        st["ps"] = pspool.tile([P, P], f32, tag="psT",
                                               name="psT")
                    ps = st["ps"]
                    nc.tensor.matmul(
                        out=ps[:],
                        lhsT=msg[:, (t - t0) * D_IN:(t - t0 + 1) * D_IN],
                        rhs=oh[:, (t - t0) * P:(t - t0 + 1) * P],
                        start=(k == 0), stop=(k == g - 1))
                    if k == g - 1:
                        nc.vector.tensor_add(
                            out=accT[:, b * P:(b + 1) * P],
                            in0=accT[:, b * P:(b + 1) * P],
                            in1=ps[:])

            qn = 0
            for w in range(cfg.NWIN):
                idx_sb = ipool.tile([P, tiles_w[w] * 8], i16, tag="idx")
                nc.sync.dma_start(out=idx_sb[:], in_=idx_t[w][:])
                dst_sb = ipool.tile([P, tiles_w[w]], f32, tag="dstb")
                nc.sync.dma_start(out=dst_sb[:], in_=dstb_t[w][:])

                tb = tile_blocks(cfg, G, w)
                for t0 in range(0, len(tb), cfg.MAX_CHUNK_TILES):
                    emit_chunk(w, t0, tb, idx_sb, dst_sb, qn)
                    qn = (qn + 1) % cfg.NQUEUES

            # ---- epilogue for blocks with no window-3 edges ----
            for b in range(nblk):
                if b not in done_blocks:
                    emit_epilogue(b)
            nc.sync.dma_start(
                out=out_t[:].rearrange("(b p) f -> p b f", p=P),
                in_=out_sb[:].rearrange("p (b f) -> p b f", f=dout))
    nc.compile()
    return nc


# ---------------------------------------------------------------- entry

def run(cfg: Cfg, input_feat, weight, cj, ci, src_idx, dst_idx, **run_kwargs):
    G, per_core, block_of = shard_edges(cfg, src_idx, dst_idx)
    shared, civs = host_inputs(cfg, input_feat, weight, cj, ci, block_of)
    nc = build_nc(cfg, G)
    in_maps = []
    for c in range(NCORES):
        m = dict(shared)
        m["civ"] = civs[c]
        m.update(per_core[c])
        in_maps.append(m)
    res = run_bass_kernel_spmd(nc, in_maps, core_ids=list(range(NCORES)),
                               **run_kwargs)
    # un-permute: core c slot s holds global dst block block_of[c, s]
    full = np.zeros((NCORES * cfg.nblk * P, cfg.D_OUT), dtype=np.float32)
    blk_rows = full.reshape(NCORES * cfg.nblk, P, cfg.D_OUT)
    for c in range(NCORES):
        o = res.results[c]["out"].reshape(cfg.nblk, P, cfg.D_OUT)
        blk_rows[block_of[c]] = o
    return full[:cfg.N], res


def kernel(input_feat, weight, cj, ci, src_idx, dst_idx):
    out, _ = run(CFG, input_feat, weight, cj, ci, src_idx, dst_idx)
    return out



# revision 2
# speedup vs baseline: 1.5681x; 1.5681x over previous
"""GCMC GraphConv kernel for 8 Trainium2 NeuronCores.

Computation:  out = ci * segment_sum((input_feat @ weight * cj)[src], dst)

Strategy v3 (aggregate-then-transform, dst-sharded, crossing-split tiles):
  - Algebra: out = ci * (A^T (cj*X)) W  — aggregate raw (cj-scaled) X rows per
    destination and multiply by W once per dst block at the end.
  - Nodes are 1D-partitioned by destination: core c owns 98 dst blocks of
    128 rows, assigned by sorted edge count for balance.
  - The per-edge gather (SWDGE dma_gather, 256B rows, 4 queues, 1024-desc
    calls) is descriptor-execution bound at ~2.6 ns/desc aggregate — so the
    dominant cost is proportional to the PADDED edge count.  v2 padded every
    (window, slot) group to a multiple of 128 (+24%).  v3 pads each group
    only to the max count over the 8 cores (+~6%) and lets matmul tiles span
    two adjacent slots: a "crossing" tile issues TWO scatter matmuls, one per
    slot, with complementary one-hot masks (rows of the other slot get
    dstb=-1 and match nothing).  The crossing structure is canonical (derived
    from the shared K[w][s] = max_c counts), so the SPMD program is identical
    across cores; only the idx/dstb data differs.
  - Scatter is a one-hot matmul accumulated TRANSPOSED into PSUM per slot,
    flushed into accT [128 feat, 98*128 dst] f32 after each (window, slot).
  - Epilogue per dst block: matmul(lhsT=accT block, rhs=W) -> [128, 64],
    scale by ci, store.
"""

import dataclasses
import math

import numpy as np
import ml_dtypes

import concourse.bacc as bacc
import concourse.mybir as mybir
import concourse.tile as tile
from concourse.bass_utils import run_bass_kernel_spmd

BF16 = ml_dtypes.bfloat16
P = 128
NCORES = 8
D_IN = 128


@dataclasses.dataclass(frozen=True)
class Cfg:
    N: int = 100000
    D_OUT: int = 64
    NWIN: int = 4            # src windows; N/NWIN must be < 32768 (int16 idx)
    MAX_CHUNK_TILES: int = 8   # gather chunk (1024 descs = SWDGE ring capacity)
    NQUEUES: int = 4         # SWDGE queues; round-robin -> concurrent rings
    SCRATCH: int = 32768     # dynamic DMA descriptor carveout (bytes/partition)

    @property
    def n_loc(self):
        return self.N // NCORES

    @property
    def nblk(self):
        return math.ceil(self.n_loc / P)

    @property
    def win(self):
        return self.N // self.NWIN


CFG = Cfg()


@dataclasses.dataclass
class Plan:
    """Canonical (core-independent) program structure."""
    off: np.ndarray        # [NWIN, nblk+1] canonical subgroup offsets
    wtot: np.ndarray       # [NWIN] padded window sizes (x128)
    ntiles: np.ndarray     # [NWIN] tiles per window
    tile_sa: list          # per window: [T_w] primary slot of each tile
    tile_cross: list       # per window: [T_w] bool, tile spans sa and sa+1
    touches: list          # per window: {slot: [(tile, 'A'|'B')]}


def make_plan(cfg: Cfg, K: np.ndarray) -> Plan:
    nblk = cfg.nblk
    off = np.zeros((cfg.NWIN, nblk + 1), dtype=np.int64)
    off[:, 1:] = np.cumsum(K, axis=1)
    wtot = ((off[:, -1] + P - 1) // P) * P
    ntiles = wtot // P
    tile_sa, tile_cross, touches = [], [], []
    for w in range(cfg.NWIN):
        assert (K[w] >= P).all(), "subgroup smaller than a tile"
        t = int(ntiles[w])
        pos0 = np.arange(t) * P
        pos1 = np.minimum(pos0 + P - 1, off[w, -1] - 1)
        sa = np.clip(np.searchsorted(off[w], pos0, side="right") - 1,
                     0, nblk - 1)
        sb = np.clip(np.searchsorted(off[w], pos1, side="right") - 1,
                     0, nblk - 1)
        assert (sb - sa <= 1).all() and (sb >= sa).all()
        cross = sb > sa
        tile_sa.append(sa)
        tile_cross.append(cross)
        tch = {s: [] for s in range(nblk)}
        for ti in range(t):
            tch[int(sa[ti])].append((ti, "A"))
            if cross[ti]:
                tch[int(sa[ti]) + 1].append((ti, "B"))
        touches.append(tch)
    return Plan(off, wtot, ntiles, tile_sa, tile_cross, touches)


# ---------------------------------------------------------------- host prep

def shard_edges(cfg: Cfg, src, dst):
    """Route and sort edges; build canonical plan + per-core padded arrays.

    Returns (plan, per_core, block_of).
    """
    nblk, win, nw = cfg.nblk, cfg.win, cfg.NWIN
    src = np.asarray(src, dtype=np.int64)
    dst = np.asarray(dst, dtype=np.int64)
    gb = dst >> 7
    dstb = (dst & 127).astype(np.float32)
    wine = src // win
    src_loc = (src - wine * win).astype(np.int16)

    nblk_g = NCORES * nblk
    bc = np.bincount(gb, minlength=nblk_g)
    order = np.argsort(-bc, kind="stable")
    block_of = np.empty((NCORES, nblk), dtype=np.int64)
    block_core = np.empty(nblk_g, dtype=np.int64)
    block_slot = np.empty(nblk_g, dtype=np.int64)
    for s in range(nblk):
        grp = order[s * NCORES:(s + 1) * NCORES]
        block_of[:, s] = grp
        block_core[grp] = np.arange(NCORES)
        block_slot[grp] = s

    core = block_core[gb]
    slot = block_slot[gb]

    gid = (core * nw + wine) * nblk + slot
    counts = np.bincount(gid, minlength=NCORES * nw * nblk)
    counts = counts.reshape(NCORES, nw, nblk)
    K = counts.max(axis=0)                    # [NWIN, nblk] canonical sizes
    plan = make_plan(cfg, K)

    per_core = []
    for c in range(NCORES):
        m = core == c
        sl, db, we, bl = src_loc[m], dstb[m], wine[m], slot[m]
        key = we * nblk + bl
        o = np.argsort(key * (win + 1) + sl, kind="stable")
        ks = key[o]
        gcnt = np.bincount(ks, minlength=nw * nblk)
        gstart = np.concatenate([[0], np.cumsum(gcnt)[:-1]])
        within = np.arange(ks.size) - gstart[ks]
        wsel, ssel = ks // nblk, ks % nblk
        pos = plan.off[wsel, ssel] + within
        sls, dbs = sl[o], db[o]
        maps = {}
        for w in range(nw):
            n = int(plan.wtot[w])
            ia = np.zeros(n, dtype=np.int16)          # pad -> row 0
            da = np.full(n, -1.0, dtype=np.float32)   # pad -> no match
            sel = wsel == w
            p, s_e, d_e = pos[sel], ssel[sel], dbs[sel]
            ia[p] = sls[sel]
            sa_of = plan.tile_sa[w][p // P]
            in_a = s_e == sa_of
            da[p[in_a]] = d_e[in_a]
            # B-stream: rows whose slot is the tile's secondary slot
            cross = plan.tile_cross[w]
            cidx = np.cumsum(cross) - 1
            ncross = int(cross.sum())
            dab = np.full(max(ncross, 1) * P, -1.0, dtype=np.float32)
            nb = ~in_a
            tt = p[nb] // P
            assert (s_e[nb] == sa_of[nb] + 1).all()
            assert cross[tt].all()
            dab[cidx[tt] * P + (p[nb] % P)] = d_e[nb]
            maps[f"idx{w}"] = np.ascontiguousarray(
                np.tile(ia.reshape(-1, 16).T, (8, 1)))
            maps[f"dstA{w}"] = np.ascontiguousarray(da.reshape(-1, P).T)
            maps[f"dstB{w}"] = np.ascontiguousarray(dab.reshape(-1, P).T)
        per_core.append(maps)
    return plan, per_core, block_of


def host_inputs(cfg: Cfg, input_feat, weight, cj, ci, block_of):
    """Shared (replicated) device inputs + per-core civ (slot layout)."""
    N, nblk = cfg.N, cfg.nblk
    xs = (np.asarray(input_feat, dtype=np.float32)
          * np.asarray(cj, dtype=np.float32)).astype(BF16)
    xw = np.ascontiguousarray(xs)
    wgt = np.ascontiguousarray(np.asarray(weight, dtype=np.float32))
    iot = np.ascontiguousarray(
        np.broadcast_to(np.arange(P, dtype=np.float32), (P, P)).astype(BF16))
    cip = np.zeros(NCORES * nblk * P, dtype=np.float32)
    cif = np.asarray(ci, dtype=np.float32).reshape(-1)
    cip[:N] = cif
    cip = cip.reshape(NCORES * nblk, P)
    civs = [np.ascontiguousarray(cip[block_of[c]].T) for c in range(NCORES)]
    return {"xw": xw, "wgt": wgt, "iot": iot}, civs


# ---------------------------------------------------------------- device IR

def build_nc(cfg: Cfg, plan: Plan):
    f32, bf16, i16 = mybir.dt.float32, mybir.dt.bfloat16, mybir.dt.int16
    dout, nblk, win = cfg.D_OUT, cfg.nblk, cfg.win
    ntiles = [int(plan.ntiles[w]) for w in range(cfg.NWIN)]
    ncross = [int(plan.tile_cross[w].sum()) for w in range(cfg.NWIN)]

    nc = bacc.Bacc("TRN2", target_bir_lowering=False, debug=False,
                   num_swdge_queues=cfg.NQUEUES,
                   dynamic_dma_scratch_size=cfg.SCRATCH)
    xw = nc.dram_tensor("xw", [cfg.N, D_IN], bf16, kind="ExternalInput")
    wgt = nc.dram_tensor("wgt", [D_IN, dout], f32, kind="ExternalInput")
    civ = nc.dram_tensor("civ", [P, nblk], f32, kind="ExternalInput")
    iot = nc.dram_tensor("iot", [P, P], bf16, kind="ExternalInput")
    idx_t = [nc.dram_tensor(f"idx{w}", [P, ntiles[w] * 8], i16,
                            kind="ExternalInput") for w in range(cfg.NWIN)]
    dsta_t = [nc.dram_tensor(f"dstA{w}", [P, ntiles[w]], f32,
                             kind="ExternalInput") for w in range(cfg.NWIN)]
    dstb_t = [nc.dram_tensor(f"dstB{w}", [P, max(ncross[w], 1)], f32,
                             kind="ExternalInput") for w in range(cfg.NWIN)]
    out_t = nc.dram_tensor("out", [nblk * P, dout], f32, kind="ExternalOutput")

    # per-window slot bookkeeping: first/last touch per slot
    first_touch = []   # per window: {(tile, which): slot}  -> start flag
    last_touch = []
    for w in range(cfg.NWIN):
        ft, lt = {}, {}
        for s, tl in plan.touches[w].items():
            ft[(tl[0][0], tl[0][1])] = s
            lt[(tl[-1][0], tl[-1][1])] = s
        first_touch.append(ft)
        last_touch.append(lt)

    with tile.TileContext(nc) as tc:
        with (
            tc.tile_pool(name="const", bufs=1) as cpool,
            tc.tile_pool(name="idx", bufs=2) as ipool,
            tc.tile_pool(name="msg", bufs=8) as mpool,
            tc.tile_pool(name="oh", bufs=8) as opool,
            tc.tile_pool(name="ohb", bufs=4) as obpool,
            tc.tile_pool(name="ps", bufs=4, space="PSUM") as pspool,
            tc.tile_pool(name="pso", bufs=4, space="PSUM") as psopool,
            tc.tile_pool(name="acc", bufs=1) as apool,
        ):
            wgt_sb = cpool.tile([P, dout], f32, tag="wgt")
            nc.sync.dma_start(out=wgt_sb[:], in_=wgt[:])
            iota_sb = cpool.tile([P, P], bf16, tag="iot")
            nc.sync.dma_start(out=iota_sb[:], in_=iot[:])
            ci_sb = cpool.tile([P, nblk], f32, tag="ci")
            nc.sync.dma_start(out=ci_sb[:], in_=civ[:])
            accT = apool.tile([P, nblk * P], f32, tag="accT")
            nc.vector.memset(accT[:], 0.0)
            out_sb = apool.tile([P, nblk * dout], f32, tag="out")

            ps_open = {}

            def emit_epilogue(s):
                pso = psopool.tile([P, dout], f32, tag="pso", name="pso")
                nc.tensor.matmul(
                    out=pso[:],
                    lhsT=accT[:, s * P:(s + 1) * P],
                    rhs=wgt_sb[:],
                    start=True, stop=True)
                nc.scalar.mul(
                    out_sb[:, s * dout:(s + 1) * dout],
                    pso[:],
                    ci_sb[:, s:s + 1])

            def emit_mm(w, t, which, s, msg, t0, oh, ohcol):
                start = first_touch[w].get((t, which)) == s
                stop = last_touch[w].get((t, which)) == s
                if start:
                    ps_open[s] = pspool.tile([P, P], f32, tag="psT",
                                             name="psT")
                ps = ps_open[s]
                nc.tensor.matmul(
                    out=ps[:],
                    lhsT=msg[:, (t - t0) * D_IN:(t - t0 + 1) * D_IN],
                    rhs=oh[:, ohcol * P:(ohcol + 1) * P],
                    start=start, stop=stop)
                if stop:
                    nc.vector.tensor_add(
                        out=accT[:, s * P:(s + 1) * P],
                        in0=accT[:, s * P:(s + 1) * P],
                        in1=ps[:])
                    del ps_open[s]
                    if w == cfg.NWIN - 1:
                        emit_epilogue(s)

            qn = 0
            for w in range(cfg.NWIN):
                tw, ncw = ntiles[w], ncross[w]
                idx_sb = ipool.tile([P, tw * 8], i16, tag="idx")
                nc.sync.dma_start(out=idx_sb[:], in_=idx_t[w][:])
                dsta_sb = ipool.tile([P, tw], f32, tag="dstA")
                nc.sync.dma_start(out=dsta_sb[:], in_=dsta_t[w][:])
                dstb_sb = ipool.tile([P, max(ncw, 1)], f32, tag="dstB")
                nc.sync.dma_start(out=dstb_sb[:], in_=dstb_t[w][:])

                sa, cross = plan.tile_sa[w], plan.tile_cross[w]
                ci_of = np.cumsum(cross) - 1
                for t0 in range(0, tw, cfg.MAX_CHUNK_TILES):
                    t1 = min(t0 + cfg.MAX_CHUNK_TILES, tw)
                    nt = t1 - t0
                    ne = nt * P
                    msg = mpool.tile([P, nt * D_IN], bf16, tag="msg")
                    nc.gpsimd.dma_gather(
                        msg[:].rearrange("p (t f) -> p t f", f=D_IN),
                        xw[w * win:(w + 1) * win, :],
                        idx_sb[:, t0 * 8:t1 * 8],
                        ne, ne, D_IN,
                        queue_num=qn)
                    qn = (qn + 1) % cfg.NQUEUES
                    oh = opool.tile([P, nt * P], bf16, tag="oh")
                    nc.vector.tensor_tensor(
                        out=oh[:].rearrange("p (t n) -> p t n", n=P),
                        in0=dsta_sb[:, t0:t1]
                            .rearrange("p (t o) -> p t o", o=1)
                            .to_broadcast([P, nt, P]),
                        in1=iota_sb[:].rearrange("p (o n) -> p o n", o=1)
                            .to_broadcast([P, nt, P]),
                        op=mybir.AluOpType.is_equal)
                    # crossing tiles in this chunk
                    cts = [t for t in range(t0, t1) if cross[t]]
                    ohb = None
                    if cts:
                        cb0, cb1 = ci_of[cts[0]], ci_of[cts[-1]] + 1
                        ncb = cb1 - cb0
                        ohb = obpool.tile([P, ncb * P], bf16, tag="ohb")
                        nc.vector.tensor_tensor(
                            out=ohb[:].rearrange("p (t n) -> p t n", n=P),
                            in0=dstb_sb[:, cb0:cb1]
                                .rearrange("p (t o) -> p t o", o=1)
                                .to_broadcast([P, ncb, P]),
                            in1=iota_sb[:].rearrange("p (o n) -> p o n", o=1)
                                .to_broadcast([P, ncb, P]),
                            op=mybir.AluOpType.is_equal)
                    for t in range(t0, t1):
                        s = int(sa[t])
                        emit_mm(w, t, "A", s, msg, t0, oh, t - t0)
                        if cross[t]:
                            emit_mm(w, t, "B", s + 1, msg, t0, ohb,
                                    int(ci_of[t]) - int(ci_of[cts[0]]))

            assert not ps_open, ps_open
            nc.sync.dma_start(
                out=out_t[:].rearrange("(b p) f -> p b f", p=P),
                in_=out_sb[:].rearrange("p (b f) -> p b f", f=dout))
    nc.compile()
    return nc


# ---------------------------------------------------------------- entry

def run(cfg: Cfg, input_feat, weight, cj, ci, src_idx, dst_idx, **run_kwargs):
    plan, per_core, block_of = shard_edges(cfg, src_idx, dst_idx)
    shared, civs = host_inputs(cfg, input_feat, weight, cj, ci, block_of)
    nc = build_nc(cfg, plan)
    in_maps = []
    for c in range(NCORES):
        m = dict(shared)
        m["civ"] = civs[c]
        m.update(per_core[c])
        in_maps.append(m)
    res = run_bass_kernel_spmd(nc, in_maps, core_ids=list(range(NCORES)),
                               **run_kwargs)
    full = np.zeros((NCORES * cfg.nblk * P, cfg.D_OUT), dtype=np.float32)
    blk_rows = full.reshape(NCORES * cfg.nblk, P, cfg.D_OUT)
    for c in range(NCORES):
        o = res.results[c]["out"].reshape(cfg.nblk, P, cfg.D_OUT)
        blk_rows[block_of[c]] = o
    return full[:cfg.N], res


def kernel(input_feat, weight, cj, ci, src_idx, dst_idx):
    out, _ = run(CFG, input_feat, weight, cj, ci, src_idx, dst_idx)
    return out
